# revision 1
# baseline (speedup 1.0000x reference)
"""EncoderG (dual-branch TAGConv encoder) as an 8-core SPMD Bass/Tile kernel
for Trainium2.

Sharding: node rows 8-way (1D row partition). Each core c owns output rows
[c*512, (c+1)*512) and holds AT_c = A.T[:, c*512:(c+1)*512] for both
adjacencies (bf16). Hop chains y_k = A @ y_{k-1} run in transposed form on
the PE — lhsT = h-chain tile slice (bf16, stationary), rhs = AT slice (bf16,
moving) — producing y_k^T [feature, local-node] in PSUM, which is exactly the
layout the (f32r) dense layers consume. The node-partition bf16 shard needed
for the inter-hop 8-core AllGather is recovered with PE transposes. The two
branches (G, L) are stage-interleaved so each branch's AllGather+reload hides
under the other branch's hop compute.

Numerics: hop-chain operands bf16 (fp32 PSUM accumulation), dense layers
float32r. BatchNorm (inference) is folded host-side into a per-feature
scale/shift applied by one ScalarE activation (fused with ReLU); conv biases
fold into the BN shift / final bias.

kernel(**inputs) takes the full unsharded inputs and returns the full
[4096, 128] output; per-core outputs are z^T shards assembled host-side.
"""
import numpy as np

N, D, H, Z, KHOPS = 4096, 512, 256, 128, 3
NCORES = 8
R = N // NCORES          # 512 local rows per core
P = 128
MT = R // P              # 4 row tiles per shard
KT = N // P              # 32 contraction tiles
GRP = 4                  # k-blocks per consolidated (DMA-batched) tile
KG = KT // GRP           # 8 big tiles
DT1 = D // P             # 4 conv1 feature tiles
HT = H // P              # 2 hidden feature tiles
EPS = 1e-3               # keras BatchNormalization epsilon

_CACHE = {}


def _build():
    import concourse.bacc as bacc
    import concourse.tile as tile
    import concourse.mybir as mybir

    F32 = mybir.dt.float32
    F32R = mybir.dt.float32r
    BF16 = mybir.dt.bfloat16
    AF = mybir.ActivationFunctionType

    nc = bacc.Bacc("TRN2", target_bir_lowering=False, debug=False,
                   num_devices=NCORES)

    at = {t: nc.dram_tensor(f"at_{t}", [N, R], BF16, kind="ExternalInput")
          for t in "GL"}
    x_bf = nc.dram_tensor("x_bf", [N, D], BF16, kind="ExternalInput")
    xt_sh = nc.dram_tensor("xt_sh", [D, R], F32R, kind="ExternalInput")
    w1 = {t: nc.dram_tensor(f"w1_{t}", [D * (KHOPS + 1), H], F32R,
                            kind="ExternalInput") for t in "GL"}
    w2 = {t: nc.dram_tensor(f"w2_{t}", [H * (KHOPS + 1), Z], F32R,
                            kind="ExternalInput") for t in "GL"}
    wm = {t: nc.dram_tensor(f"wm_{t}", [H, Z], F32R, kind="ExternalInput")
          for t in "GL"}
    bn_sc = {t: nc.dram_tensor(f"bn_sc_{t}", [H, 1], F32, kind="ExternalInput")
             for t in "GL"}
    bn_sh = {t: nc.dram_tensor(f"bn_sh_{t}", [H, 1], F32, kind="ExternalInput")
             for t in "GL"}
    zbias = nc.dram_tensor("zbias", [Z, 1], F32, kind="ExternalInput")
    ident = nc.dram_tensor("ident", [P, P], F32R, kind="ExternalInput")
    out_t = nc.dram_tensor("out_t", [Z, R], F32, kind="ExternalOutput")

    RG = [list(range(NCORES))]

    def grp_ap(dram_ap, g, rows_per_grp):
        return dram_ap[g * rows_per_grp:(g + 1) * rows_per_grp, :].rearrange(
            "(b p) d -> p b d", p=P)

    with tile.TileContext(nc) as tc:
        with (
            tc.tile_pool(name="atp", bufs=KG) as atp,
            tc.tile_pool(name="chainp", bufs=KG) as chainp,
            tc.tile_pool(name="wp", bufs=2) as wp,
            tc.tile_pool(name="ysp", bufs=2) as ysp,
            tc.tile_pool(name="h1tp", bufs=2) as h1tp,
            tc.tile_pool(name="smallp", bufs=2) as smallp,
            tc.tile_pool(name="hop_ps", bufs=3, space="PSUM") as hop_ps,
            tc.tile_pool(name="acc1_ps", bufs=4, space="PSUM") as acc1_ps,
            tc.tile_pool(name="acc2_ps", bufs=1, space="PSUM") as acc2_ps,
            tc.tile_pool(name="agin", bufs=2, space="DRAM") as agin,
            tc.tile_pool(name="agout", bufs=2, space="DRAM") as agout,
        ):
            dma_rr = [0]

            def dma(out_ap, in_ap):
                # alternate the two HWDGE rings (SP / ACT) for parallelism
                eng = (nc.sync, nc.scalar)[dma_rr[0] % 2]
                dma_rr[0] += 1
                eng.dma_start(out_ap, in_ap)

            ident_t = smallp.tile([P, P], F32R, name="ident", tag="ident")
            dma(ident_t[:], ident[:])
            zbias_t = smallp.tile([Z, 1], F32, name="zbias", tag="zb")
            dma(zbias_t[:], zbias[:])

            state = {}
            ACC2_TOTAL = 2 * ((KHOPS + 1) * HT + HT)

            def acc2_mm(lhsT, rhs):
                nc.tensor.matmul(state["acc2"][:], lhsT, rhs,
                                 start=(state["n"] == 0),
                                 stop=(state["n"] == ACC2_TOTAL - 1))
                state["n"] += 1

            def transpose_to_bf16(dst_ap, src_ap, name):
                tp = hop_ps.tile([P, P], F32R, name=name, tag="hop")
                nc.tensor.transpose(tp[:], src_ap, ident_t[:])
                nc.vector.tensor_copy(dst_ap, tp[:])

            def allgather(shard_big, width, tag, branch_tag):
                bounce_in = agin.tile([R, width], BF16, name=f"agi_{tag}",
                                      tag="agin")
                dma(bounce_in[:].rearrange("(b p) d -> p b d", p=P),
                    shard_big[:].rearrange("p (b d) -> p b d", b=MT))
                bounce_out = agout.tile([N, width], BF16, name=f"ago_{tag}",
                                        tag="agout", addr_space="Shared")
                nc.gpsimd.collective_compute(
                    "AllGather", mybir.AluOpType.bypass, replica_groups=RG,
                    ins=[bounce_in.opt()], outs=[bounce_out.opt()])
                tiles = []
                for g in range(KG):
                    t = chainp.tile([P, GRP * width], BF16, name=f"h_{tag}_{g}",
                                    tag=f"chain{branch_tag}")
                    dma(t[:].rearrange("p (b d) -> p b d", b=GRP),
                        grp_ap(bounce_out, g, GRP * P))
                    tiles.append(t)
                return tiles

            def hop_matmuls(h_tiles, at_t, width, name):
                ndt = width // P
                yts = ysp.tile([P, ndt * R], F32R, name=f"{name}_yts", tag="ys")
                for d0 in range(0, ndt, 2):
                    dts = range(d0, min(d0 + 2, ndt))
                    yt_ps = {dt: hop_ps.tile([P, R], F32, name=f"{name}_ps{dt}",
                                             tag="hop") for dt in dts}
                    for k in range(KT):
                        g, b = k // GRP, k % GRP
                        rhs = at_t[g][:, b * R:(b + 1) * R]
                        for dt in dts:
                            lhsT = h_tiles[g][:, b * width + dt * P:
                                              b * width + (dt + 1) * P]
                            nc.tensor.matmul(yt_ps[dt][:], lhsT, rhs,
                                             start=(k == 0), stop=(k == KT - 1))
                    for dt in dts:
                        nc.vector.tensor_copy(yts[:, dt * R:(dt + 1) * R],
                                              yt_ps[dt][:])
                return yts

            def to_node_shard(yts, width, name):
                ndt = width // P
                shard = ysp.tile([P, MT * width], BF16, name=f"{name}_sh",
                                 tag="ys")
                for m in range(MT):
                    for dt in range(ndt):
                        transpose_to_bf16(
                            shard[:, m * width + dt * P:m * width + (dt + 1) * P],
                            yts[:, dt * R + m * P:dt * R + (m + 1) * P],
                            f"{name}_tp{m}_{dt}")
                return shard

            def branch(tag):
                if tag == "G":
                    at_t = state["atG_tiles"]
                else:
                    at_t = []
                    for g in range(KG):
                        t = atp.tile([P, GRP * R], BF16, name=f"at{tag}_{g}",
                                     tag=f"at{tag}")
                        dma(t[:].rearrange("p (b d) -> p b d", b=GRP),
                            grp_ap(at[tag], g, GRP * P))
                        at_t.append(t)
                h_tiles = state["x_tiles"]
                w1a_t = wp.tile([P, DT1 * H], F32R,
                               name=f"w1{tag}" + "a", tag="w1a")
                dma(w1a_t[:].rearrange("p (b d) -> p b d", p=P, d=H),
                    w1[tag][:DT1 * P, :].rearrange("(b p) d -> p b d", p=P))
                w1b_t = wp.tile([P, KHOPS * DT1 * H], F32R,
                               name=f"w1{tag}" + "b", tag="w1b")
                dma(w1b_t[:].rearrange("p (b d) -> p b d", p=P, d=H),
                    w1[tag][DT1 * P:, :].rearrange("(b p) d -> p b d", p=P))
                w2_t = wp.tile([P, (KHOPS + 1) * HT * Z], F32R,
                               name=f"w2{tag}", tag="w2")
                dma(w2_t[:].rearrange("p (b d) -> p b d", p=P, d=Z),
                    w2[tag][:].rearrange("(b p) d -> p b d", p=P))
                wm_t = wp.tile([P, HT * Z], F32R, name=f"wm{tag}", tag="wm")
                dma(wm_t[:].rearrange("p (b d) -> p b d", p=P, d=Z),
                    wm[tag][:].rearrange("(b p) d -> p b d", p=P))
                bn_sc_t = smallp.tile([P, HT], F32, name=f"bnsc{tag}",
                                      tag="bn1")
                dma(bn_sc_t[:].rearrange("p (b d) -> p b d", p=P, d=1),
                    bn_sc[tag][:].rearrange("(b p) d -> p b d", p=P))
                bn_sh_t = smallp.tile([P, HT], F32, name=f"bnsh{tag}",
                                      tag="bn2")
                dma(bn_sh_t[:].rearrange("p (b d) -> p b d", p=P, d=1),
                    bn_sh[tag][:].rearrange("(b p) d -> p b d", p=P))
                xt_t = state["xt_t"]

                def w1_slice(khop, dt, hf):
                    if khop == 0:
                        base = dt * H + hf * P
                        return w1a_t[:, base:base + P]
                    base = ((khop - 1) * DT1 + dt) * H + hf * P
                    return w1b_t[:, base:base + P]

                def w2_slice(khop, dt):
                    base = (khop * HT + dt) * Z
                    return w2_t[:, base:base + Z]


                acc1 = [acc1_ps.tile([P, R], F32, name=f"acc1{tag}_{hf}",
                                     tag="acc1") for hf in range(HT)]
                n1 = DT1 * (KHOPS + 1)
                cnt1 = [0, 0]

                def dense1_mm(hf, lhsT, rhs):
                    nc.tensor.matmul(acc1[hf][:], lhsT, rhs,
                                     start=(cnt1[hf] == 0),
                                     stop=(cnt1[hf] == n1 - 1))
                    cnt1[hf] += 1

                for khop in range(1, KHOPS + 1):
                    yts = hop_matmuls(h_tiles, at_t, D, f"y{tag}{khop}")
                    if khop < KHOPS:
                        shard = to_node_shard(yts, D, f"y{tag}{khop}")
                        h_tiles = allgather(shard, D, f"{tag}1_{khop}", tag)
                    for dt in range(DT1):
                        for hf in range(HT):
                            dense1_mm(hf, w1_slice(khop, dt, hf),
                                      yts[:, dt * R:(dt + 1) * R])
                    if khop == 1:
                        for dt in range(DT1):
                            for hf in range(HT):
                                dense1_mm(hf, w1_slice(0, dt, hf),
                                          xt_t[:, dt * R:(dt + 1) * R])
                    if khop < KHOPS:
                        yield

                h1t = []
                for hf in range(HT):
                    t = h1tp.tile([P, R], F32R, name=f"h1t{tag}_{hf}",
                                  tag="h1t")
                    nc.scalar.activation(t[:], acc1[hf][:], AF.Relu,
                                         bias=bn_sh_t[:, hf:hf + 1],
                                         scale=bn_sc_t[:, hf:hf + 1])
                    h1t.append(t)

                for dt in range(HT):
                    acc2_mm(w2_slice(0, dt), h1t[dt][:])
                for dt in range(HT):
                    acc2_mm(wm_t[:, dt * Z:(dt + 1) * Z], h1t[dt][:])

                h1ts = ysp.tile([P, HT * R], F32R, name=f"h1ts{tag}", tag="ys")
                for hf in range(HT):
                    nc.vector.tensor_copy(h1ts[:, hf * R:(hf + 1) * R],
                                          h1t[hf][:])
                shard = to_node_shard(h1ts, H, f"h1{tag}")
                h_tiles = allgather(shard, H, f"{tag}2_0", tag)
                yield

                for khop in range(1, KHOPS + 1):
                    yts = hop_matmuls(h_tiles, at_t, H, f"z{tag}{khop}")
                    for dt in range(HT):
                        acc2_mm(w2_slice(khop, dt), yts[:, dt * R:(dt + 1) * R])
                    if khop < KHOPS:
                        shard = to_node_shard(yts, H, f"z{tag}{khop}")
                        h_tiles = allgather(shard, H, f"{tag}2_{khop}", tag)
                        yield

            state["acc2"] = acc2_ps.tile([P, R], F32, name="acc2", tag="acc2")
            state["n"] = 0
            x_tiles = []
            atG_tiles = []
            for g in range(KG):
                a = atp.tile([P, GRP * R], BF16, name=f"atG_{g}", tag="atG")
                dma(a[:].rearrange("p (b d) -> p b d", b=GRP),
                    grp_ap(at["G"], g, GRP * P))
                atG_tiles.append(a)
                t = chainp.tile([P, GRP * D], BF16, name=f"x_{g}",
                                tag="chainG")
                dma(t[:].rearrange("p (b d) -> p b d", b=GRP),
                    grp_ap(x_bf, g, GRP * P))
                x_tiles.append(t)
            state["atG_tiles"] = atG_tiles
            state["x_tiles"] = x_tiles
            xt_t = ysp.tile([P, DT1 * R], F32R, name="xt", tag="xt", bufs=1)
            dma(xt_t[:].rearrange("p (b d) -> p b d", b=DT1),
                xt_sh[:].rearrange("(b p) d -> p b d", p=P))
            state["xt_t"] = xt_t

            gens = [branch("G"), branch("L")]
            done = [False, False]
            while not all(done):
                for i, g in enumerate(gens):
                    if not done[i]:
                        try:
                            next(g)
                        except StopIteration:
                            done[i] = True

            out_sb = ysp.tile([Z, R], F32, name="out_sb", tag="ys")
            nc.vector.tensor_scalar_add(out_sb[:], state["acc2"][:],
                                        zbias_t[:])
            dma(out_t[:], out_sb[:])

    nc.compile()
    return nc


def _make_in_maps(inputs):
    import ml_dtypes
    bf16 = ml_dtypes.bfloat16
    x = np.asarray(inputs["x"], np.float32)
    at_full = {t: np.ascontiguousarray(
        np.asarray(inputs[f"A_{t}"], np.float32).T.astype(bf16))
        for t in "GL"}
    prep = {}
    for t in "GL":
        g = np.asarray(inputs[f"gamma_{t}"], np.float32)
        b = np.asarray(inputs[f"beta_{t}"], np.float32)
        mu = np.asarray(inputs[f"mean_{t}"], np.float32)
        v = np.asarray(inputs[f"var_{t}"], np.float32)
        b1 = np.asarray(inputs[f"b1_{t}"], np.float32)
        sc = g / np.sqrt(v + EPS)
        sh = (b1 - mu) * sc + b
        prep[f"bn_sc_{t}"] = np.ascontiguousarray(sc.reshape(H, 1))
        prep[f"bn_sh_{t}"] = np.ascontiguousarray(sh.reshape(H, 1))
        prep[f"w1_{t}"] = np.ascontiguousarray(inputs[f"W1_{t}"], np.float32)
        prep[f"w2_{t}"] = np.ascontiguousarray(inputs[f"W2_{t}"], np.float32)
        prep[f"wm_{t}"] = np.ascontiguousarray(inputs[f"Wm_{t}"], np.float32)
    zb = sum(np.asarray(inputs[f"b2_{t}"], np.float32) +
             np.asarray(inputs[f"bm_{t}"], np.float32) for t in "GL")
    prep["zbias"] = np.ascontiguousarray(zb.reshape(Z, 1))
    prep["ident"] = np.eye(P, dtype=np.float32)
    prep["x_bf"] = np.ascontiguousarray(x.astype(bf16))
    in_maps = []
    for c in range(NCORES):
        sl = slice(c * R, (c + 1) * R)
        m = dict(prep)
        m["xt_sh"] = np.ascontiguousarray(x[sl].T)
        for t in "GL":
            m[f"at_{t}"] = np.ascontiguousarray(at_full[t][:, sl])
        in_maps.append(m)
    return in_maps


def _get_nc():
    if "nc" not in _CACHE:
        _CACHE["nc"] = _build()
    return _CACHE["nc"]


def kernel(**inputs) -> np.ndarray:
    from concourse.bass_utils import run_bass_kernel_spmd

    nc = _get_nc()
    in_maps = _make_in_maps(inputs)
    res = run_bass_kernel_spmd(nc, in_maps, list(range(NCORES)))
    out = np.empty((N, Z), np.float32)
    for c in range(NCORES):
        out[c * R:(c + 1) * R, :] = res.results[c]["out_t"].T
    return out



# revision 2
# speedup vs baseline: 1.7192x; 1.7192x over previous
"""EncoderG via Horner-form TAGConv — 8-core SPMD Bass/Tile kernel.

Math: concat([x, Ax, A^2x, A^3x]) @ W  ==  c_0 + A(c_1 + A(c_2 + A c_3))
with c_k = x @ W_k (associativity). Hop chains therefore run at the
OUTPUT width (H=256 for conv1, Z=128 for conv2) instead of the input
width — 2x fewer hop FLOPs — and the per-hop AllGather moves the
narrower u-vectors instead of the wide feature chains.

Sharding: node rows 8-way. Core c holds AT = A.T[:, c*512:(c+1)*512]
(both adjacencies). Levels: u_3 = c_3; u_k = c_k + A u_{k+1}. Dense
c_k parts contract over local x^T (d-major); hop parts contract over
global nodes with the gathered u (node-major bf16/fp8) as stationary
and AT as moving, producing u^T [feature, local] in PSUM — dense and
hop accumulate into the same PSUM tile. Node-major shards for the
AllGather are recovered with PE transposes.

fp8 mode: chain operands (AT, gathered u) are float8e4 with A scaled by
4096 (entries -> [0,1]); all dense weights carry the same x4096 scale so
every accumulator holds 4096*value; BN scale and the final bias-add
fold the 1/4096 back out. Hop matmuls use DoubleRow perf mode (2
k-tiles per instruction, 2x PE rate).
"""
import numpy as np

N, D, H, Z, KHOPS = 4096, 512, 256, 128, 3
NCORES = 8
R = N // NCORES          # 512 local rows per core
P = 128
MT = R // P              # 4 row blocks per shard
KT = N // P              # 32 contraction tiles
GRP = 4                  # k-tiles per consolidated (DMA-batched) tile
KG = KT // GRP           # 8 big tiles
DT1 = D // P             # 4 x^T feature tiles
HT = H // P              # 2 conv1 width tiles
EPS = 1e-3


def build(T: int = 1, chain_fp8: bool = True):
    import concourse.bacc as bacc
    import concourse.tile as tile
    import concourse.mybir as mybir

    F32 = mybir.dt.float32
    BF16 = mybir.dt.bfloat16
    CDT = mybir.dt.float8e4 if chain_fp8 else BF16
    AF = mybir.ActivationFunctionType
    DR = mybir.MatmulPerfMode.DoubleRow
    INV_SCALE = (1.0 / 4096.0) if chain_fp8 else 1.0

    nc = bacc.Bacc("TRN2", target_bir_lowering=False, debug=False,
                   num_devices=NCORES)

    at = {t: nc.dram_tensor(f"at_{t}", [N, R], CDT, kind="ExternalInput")
          for t in "GL"}
    xt_sh = nc.dram_tensor("xt_sh", [D, R], BF16, kind="ExternalInput")
    w1 = {t: nc.dram_tensor(f"w1_{t}", [D * (KHOPS + 1), H], BF16,
                            kind="ExternalInput") for t in "GL"}
    w2 = {t: nc.dram_tensor(f"w2_{t}", [H * (KHOPS + 1), Z], BF16,
                            kind="ExternalInput") for t in "GL"}
    wm = {t: nc.dram_tensor(f"wm_{t}", [H, Z], BF16, kind="ExternalInput")
          for t in "GL"}
    bn_sc = {t: nc.dram_tensor(f"bn_sc_{t}", [H, 1], F32, kind="ExternalInput")
             for t in "GL"}
    bn_sh = {t: nc.dram_tensor(f"bn_sh_{t}", [H, 1], F32, kind="ExternalInput")
             for t in "GL"}
    zbias = nc.dram_tensor("zbias", [Z, 1], F32, kind="ExternalInput")
    ident = nc.dram_tensor("ident", [P, P], BF16, kind="ExternalInput")
    out_t = nc.dram_tensor("out_t", [Z, R], F32, kind="ExternalOutput")

    RG = [list(range(NCORES))]

    def grp_ap(dram_ap, g, rows_per_grp):
        return dram_ap[g * rows_per_grp:(g + 1) * rows_per_grp, :].rearrange(
            "(b p) d -> p b d", p=P)

    with tile.TileContext(nc) as tc:
        with (
            tc.tile_pool(name="atp", bufs=KG) as atp,
            tc.tile_pool(name="chainp", bufs=2 * KG) as chainp,
            tc.tile_pool(name="wp", bufs=1) as wp,
            tc.tile_pool(name="ucp", bufs=4) as ucp,
            tc.tile_pool(name="shardp", bufs=4) as shardp,
            tc.tile_pool(name="h1tp", bufs=2) as h1tp,
            tc.tile_pool(name="smallp", bufs=1) as smallp,
            tc.tile_pool(name="outp", bufs=1) as outp,
            tc.tile_pool(name="ups", bufs=5, space="PSUM") as ups,
            tc.tile_pool(name="acc2p", bufs=1, space="PSUM") as acc2p,
            tc.tile_pool(name="tpp", bufs=2, space="PSUM") as tpp,
            tc.tile_pool(name="agin", bufs=4, space="DRAM") as agin,
            tc.tile_pool(name="agout", bufs=4, space="DRAM") as agout,
        ):
          for _it in range(T):
            dma_rr = [0]

            def dma(out_ap, in_ap):
                # alternate the two HWDGE rings (SP / ACT) for parallelism
                eng = (nc.sync, nc.scalar)[dma_rr[0] % 2]
                dma_rr[0] += 1
                eng.dma_start(out_ap, in_ap)

            ident_t = smallp.tile([P, P], BF16, name="ident", tag="ident")
            dma(ident_t[:], ident[:])
            zbias_t = smallp.tile([Z, 1], F32, name="zbias", tag="zb")
            dma(zbias_t[:], zbias[:])
            xt_t = smallp.tile([P, DT1 * R], BF16, name="xt", tag="xt")
            dma(xt_t[:].rearrange("p (b d) -> p b d", b=DT1),
                xt_sh[:].rearrange("(b p) d -> p b d", p=P))

            state = {"n2": 0}
            HOPN = KT // 2 if chain_fp8 else KT
            ACC2_TOTAL = 2 * (2 * HT + HOPN)

            def load_at(tag):
                tiles = []
                for g in range(KG):
                    a = atp.tile([P, GRP * R], CDT, name=f"at{tag}_{g}",
                                 tag=f"at{tag}")
                    dma(a[:].rearrange("p (b d) -> p b d", b=GRP),
                        grp_ap(at[tag], g, GRP * P))
                    tiles.append(a)
                return tiles

            def hop_mms(acc_ap, ug, at_t, hf, width, start, stop):
                """Accumulate (A u)^T[hf-block, local] into acc_ap."""
                if chain_fp8:
                    for j in range(KT // 2):
                        g, b0 = (2 * j) // GRP, (2 * j) % GRP
                        lhsT = ug[g][:].rearrange(
                            "p (b w) -> p b w", b=GRP)[:, b0:b0 + 2,
                                                       hf * P:(hf + 1) * P]
                        rhs = at_t[g][:].rearrange(
                            "p (b r) -> p b r", b=GRP)[:, b0:b0 + 2, :]
                        nc.tensor.matmul(acc_ap, lhsT, rhs,
                                         start=(start and j == 0),
                                         stop=(stop and j == KT // 2 - 1),
                                         perf_mode=DR)
                else:
                    for k in range(KT):
                        g, b = k // GRP, k % GRP
                        lhsT = ug[g][:, b * width + hf * P:
                                     b * width + (hf + 1) * P]
                        rhs = at_t[g][:, b * R:(b + 1) * R]
                        nc.tensor.matmul(acc_ap, lhsT, rhs,
                                         start=(start and k == 0),
                                         stop=(stop and k == KT - 1))

            def make_shard(accs, width, name):
                # transposes run in bf16 (walrus rejects fp8 PE transpose);
                # the fp8 quantization happens in the tp->shard copy.
                nw = width // P
                shard = shardp.tile([P, MT * width], CDT, name=f"{name}_sh",
                                    tag="shard")
                for hf in range(nw):
                    uc = ucp.tile([P, R], BF16, name=f"{name}_uc{hf}",
                                  tag="uc")
                    if chain_fp8:
                        nc.scalar.activation(uc[:], accs[hf][:], AF.Copy,
                                             scale=INV_SCALE)
                    else:
                        nc.vector.tensor_copy(uc[:], accs[hf][:])
                    for m in range(MT):
                        tp = tpp.tile([P, P], BF16, name=f"{name}_tp{hf}_{m}",
                                      tag="tp")
                        nc.tensor.transpose(tp[:], uc[:, m * P:(m + 1) * P],
                                            ident_t[:])
                        nc.vector.tensor_copy(
                            shard[:, m * width + hf * P:m * width +
                                  (hf + 1) * P], tp[:])
                return shard

            def allgather(shard, width, tag, branch_tag):
                bounce_in = agin.tile([R, width], CDT, name=f"agi_{tag}",
                                      tag="agin")
                dma(bounce_in[:].rearrange("(b p) d -> p b d", p=P),
                    shard[:].rearrange("p (b d) -> p b d", b=MT))
                bounce_out = agout.tile([N, width], CDT, name=f"ago_{tag}",
                                        tag="agout", addr_space="Shared")
                nc.gpsimd.collective_compute(
                    "AllGather", mybir.AluOpType.bypass, replica_groups=RG,
                    ins=[bounce_in.opt()], outs=[bounce_out.opt()])
                tiles = []
                for g in range(KG):
                    t = chainp.tile([P, GRP * width], CDT, name=f"u_{tag}_{g}",
                                    tag=f"chain{branch_tag}")
                    dma(t[:].rearrange("p (b d) -> p b d", b=GRP),
                        grp_ap(bounce_out, g, GRP * P))
                    tiles.append(t)
                return tiles

            def load_w1(tag):
                w1_t = wp.tile([P, (KHOPS + 1) * DT1 * H], BF16,
                               name=f"w1{tag}", tag=f"w1{tag}")
                for k in range(KHOPS, -1, -1):   # k=3 slice first
                    dma(w1_t[:, k * DT1 * H:(k + 1) * DT1 * H].rearrange(
                            "p (b d) -> p b d", d=H),
                        w1[tag][k * D:(k + 1) * D, :].rearrange(
                            "(b p) d -> p b d", p=P))
                return w1_t

            def branch(tag):
                w1_t = state[f"w1{tag}"]
                w2_t = wp.tile([P, (KHOPS + 1) * HT * Z], BF16,
                               name=f"w2{tag}", tag=f"w2{tag}")
                dma(w2_t[:].rearrange("p (b d) -> p b d", d=Z),
                    w2[tag][:].rearrange("(b p) d -> p b d", p=P))
                wm_t = wp.tile([P, HT * Z], BF16, name=f"wm{tag}",
                               tag=f"wm{tag}")
                dma(wm_t[:].rearrange("p (b d) -> p b d", d=Z),
                    wm[tag][:].rearrange("(b p) d -> p b d", p=P))
                bn_sc_t = smallp.tile([P, HT], F32, name=f"bnsc{tag}",
                                      tag=f"bn1{tag}")
                dma(bn_sc_t[:].rearrange("p (b d) -> p b d", d=1),
                    bn_sc[tag][:].rearrange("(b p) d -> p b d", p=P))
                bn_sh_t = smallp.tile([P, HT], F32, name=f"bnsh{tag}",
                                      tag=f"bn2{tag}")
                dma(bn_sh_t[:].rearrange("p (b d) -> p b d", d=1),
                    bn_sh[tag][:].rearrange("(b p) d -> p b d", p=P))

                def w1_slice(k, dt, hf):
                    base = (k * DT1 + dt) * H + hf * P
                    return w1_t[:, base:base + P]

                def w2_slice(k, ht):
                    base = (k * HT + ht) * Z
                    return w2_t[:, base:base + Z]

                def dense1(accs, k, stop):
                    for hf in range(HT):
                        for dt in range(DT1):
                            nc.tensor.matmul(
                                accs[hf][:], w1_slice(k, dt, hf),
                                xt_t[:, dt * R:(dt + 1) * R],
                                start=(dt == 0),
                                stop=(stop and dt == DT1 - 1))

                def new_accs(nm, n):
                    return [ups.tile([P, R], F32, name=f"{nm}{tag}_{hf}",
                                     tag="u") for hf in range(n)]

                # ---- conv1: u_3 = c_3; u_k = c_k + A u_{k+1}; y = u_0 ----
                accs = new_accs("u3", HT)
                dense1(accs, KHOPS, stop=True)
                ug = allgather(make_shard(accs, H, f"u3{tag}"), H,
                               f"{tag}u3", tag)
                at_t = load_at(tag)
                yield
                for k in range(KHOPS - 1, -1, -1):
                    accs = new_accs(f"u{k}", HT)
                    dense1(accs, k, stop=False)
                    yield
                    for hf in range(HT):
                        hop_mms(accs[hf][:], ug, at_t, hf, H,
                                start=False, stop=True)
                    if k > 0:
                        ug = allgather(make_shard(accs, H, f"u{k}{tag}"), H,
                                       f"{tag}u{k}", tag)
                        yield

                # ---- BN + ReLU ----
                h1t = []
                for hf in range(HT):
                    t = h1tp.tile([P, R], BF16, name=f"h1t{tag}_{hf}",
                                  tag=f"h1{tag}")
                    nc.scalar.activation(t[:], accs[hf][:], AF.Relu,
                                         bias=bn_sh_t[:, hf:hf + 1],
                                         scale=bn_sc_t[:, hf:hf + 1])
                    h1t.append(t)

                # ---- conv2: v_3 = d_3; v_k = d_k + A v_{k+1} ----
                state["w"] = Z

                def dense2(acc_ap, k, start, stop):
                    for ht in range(HT):
                        nc.tensor.matmul(acc_ap, w2_slice(k, ht),
                                         h1t[ht][:], start=(start and ht == 0),
                                         stop=(stop and ht == HT - 1))

                acc_v = new_accs("v3", 1)
                dense2(acc_v[0][:], KHOPS, start=True, stop=True)
                vg = allgather(make_shard(acc_v, Z, f"v3{tag}"), Z,
                               f"{tag}v3", tag)
                yield
                for k in range(KHOPS - 1, 0, -1):
                    acc_v = new_accs(f"v{k}", 1)
                    dense2(acc_v[0][:], k, start=True, stop=False)
                    yield
                    hop_mms(acc_v[0][:], vg, at_t, 0, start=False, stop=True)
                    vg = allgather(make_shard(acc_v, Z, f"v{k}{tag}"), Z,
                                   f"{tag}v{k}", tag)
                    yield

                # ---- level 0 into the shared z accumulator ----
                def acc2_mm(lhsT, rhs, perf_mode=None):
                    nc.tensor.matmul(state["acc2"][:], lhsT, rhs,
                                     start=(state["n2"] == 0),
                                     stop=(state["n2"] == ACC2_TOTAL - 1),
                                     perf_mode=perf_mode,
                                     skip_group_check=True)
                    state["n2"] += 1

                for ht in range(HT):
                    acc2_mm(w2_slice(0, ht), h1t[ht][:])
                for ht in range(HT):
                    acc2_mm(wm_t[:, ht * Z:(ht + 1) * Z], h1t[ht][:])
                yield
                if chain_fp8:
                    for j in range(KT // 2):
                        g, b0 = (2 * j) // GRP, (2 * j) % GRP
                        lhsT = vg[g][:].rearrange(
                            "p (b w) -> p b w", b=GRP)[:, b0:b0 + 2, :]
                        rhs = at_t[g][:].rearrange(
                            "p (b r) -> p b r", b=GRP)[:, b0:b0 + 2, :]
                        acc2_mm(lhsT, rhs, perf_mode=DR)
                else:
                    for k in range(KT):
                        g, b = k // GRP, k % GRP
                        acc2_mm(vg[g][:, b * Z:(b + 1) * Z],
                                at_t[g][:, b * R:(b + 1) * R])

            state["acc2"] = acc2p.tile([P, R], F32, name="acc2", tag="acc2")
            state["atG_tiles"] = load_at("G")

            gens = [branch("G"), branch("L")]
            done = [False, False]
            while not all(done):
                for i, g in enumerate(gens):
                    if not done[i]:
                        try:
                            next(g)
                        except StopIteration:
                            done[i] = True

            out_sb = outp.tile([Z, R], F32, name="out_sb", tag="out")
            nc.scalar.activation(out_sb[:], state["acc2"][:], AF.Identity,
                                 bias=zbias_t[:], scale=INV_SCALE)
            dma(out_t[:], out_sb[:])

    nc.compile()
    return nc


def make_in_maps(inputs, chain_fp8: bool = True):
    import ml_dtypes
    bf16 = ml_dtypes.bfloat16
    cdt = ml_dtypes.float8_e4m3fn if chain_fp8 else bf16
    SCALE = 4096.0 if chain_fp8 else 1.0
    x = np.asarray(inputs["x"], np.float32)
    at_full = {t: np.ascontiguousarray(
        (np.asarray(inputs[f"A_{t}"], np.float32).T * SCALE).astype(cdt))
        for t in "GL"}
    prep = {}
    for t in "GL":
        g = np.asarray(inputs[f"gamma_{t}"], np.float32)
        b = np.asarray(inputs[f"beta_{t}"], np.float32)
        mu = np.asarray(inputs[f"mean_{t}"], np.float32)
        v = np.asarray(inputs[f"var_{t}"], np.float32)
        b1 = np.asarray(inputs[f"b1_{t}"], np.float32)
        sc = g / np.sqrt(v + EPS)
        sh = (b1 - mu) * sc + b
        prep[f"bn_sc_{t}"] = np.ascontiguousarray((sc / SCALE).reshape(H, 1))
        prep[f"bn_sh_{t}"] = np.ascontiguousarray(sh.reshape(H, 1))
        for nm, hh in [("w1", "W1"), ("w2", "W2"), ("wm", "Wm")]:
            prep[f"{nm}_{t}"] = np.ascontiguousarray(
                (np.asarray(inputs[f"{hh}_{t}"], np.float32) * SCALE
                 ).astype(bf16))
    zb = sum(np.asarray(inputs[f"b2_{t}"], np.float32) +
             np.asarray(inputs[f"bm_{t}"], np.float32) for t in "GL")
    prep["zbias"] = np.ascontiguousarray(zb.reshape(Z, 1))
    prep["ident"] = np.eye(P, dtype=np.float32).astype(bf16)
    in_maps = []
    for c in range(NCORES):
        sl = slice(c * R, (c + 1) * R)
        m = dict(prep)
        m["xt_sh"] = np.ascontiguousarray(x[sl].T.astype(bf16))
        for t in "GL":
            m[f"at_{t}"] = np.ascontiguousarray(at_full[t][:, sl])
        in_maps.append(m)
    return in_maps


_CACHE = {}


def _get_nc():
    if "nc" not in _CACHE:
        _CACHE["nc"] = build(T=1, chain_fp8=True)
    return _CACHE["nc"]


def kernel(**inputs) -> np.ndarray:
    from concourse.bass_utils import run_bass_kernel_spmd

    nc = _get_nc()
    in_maps = make_in_maps(inputs, chain_fp8=True)
    res = run_bass_kernel_spmd(nc, in_maps, list(range(NCORES)))
    out = np.empty((N, Z), np.float32)
    for c in range(NCORES):
        out[c * R:(c + 1) * R, :] = res.results[c]["out_t"].T
    return out
